# revision 72
# baseline (speedup 1.0000x reference)
"""GCN encoder (3x GCNConv+BN, mean-pool) on 8 Trainium2 NeuronCores.

Sharding: nodes are permuted and dealt into 8 shards (SH rows each incl.
dummy padding). Core c = (pair p = c%4, source-half h = c//4) aggregates the
edges with dst in shards {p, p+4} and src in half h (halves = shards 0-3 /
4-7, 4*SH rows each, so gather indices fit int16 for dma_gather).
ReduceScatter over pairs [[0,4],[1,5],[2,6],[3,7]] sums the two partial
aggregations; AllGather over [[0,1,2,3],[4,5,6,7]] rebuilds each half's
gather table after every layer's linear transform.

Norm folding: norm(e) = dinv[src]*dinv[dst] is factorized — the gather table
stores z*dinv[row] and the dst factor is applied once after ReduceScatter.
Conv biases cancel inside BatchNorm; BN itself is a per-channel affine fused
into a single scalar-engine activation (scale+bias+relu) applied to the
PE-transposed tiles. Layer 3's BN affine commutes with mean-pooling and is
applied once to the final pooled [64, G'] tensor.
"""

import os
import numpy as np

D = 64
EPS = 1e-5
NCORES = 8
SLOTS_PER_INST = 1024
CHUNKS_PER_INST = SLOTS_PER_INST // 128  # 8


def make_cfg(N, G, SHT):
    cfg = {}
    cfg["N"] = N
    cfg["G"] = G
    cfg["SHT"] = SHT
    cfg["SH"] = SHT * 128
    cfg["HALF"] = 4 * cfg["SH"]
    cfg["NPAD"] = 8 * cfg["SH"]
    cfg["NREAL_SH"] = N // NCORES
    assert N % NCORES == 0 and cfg["NREAL_SH"] < cfg["SH"]
    cfg["PADROW"] = cfg["NREAL_SH"]
    cfg["NCHUNK"] = max(1, -(-(G + 1) // 128))
    cfg["PADG"] = cfg["NCHUNK"] * 128 - 1
    cfg["NT"] = 2 * SHT
    # AllGather chunk boundaries (in tiles); table is laid out chunk-major
    # ([chunk][shard][tile rows]) so each chunked AllGather output is
    # contiguous
    AGC = 2
    cuts = [0]
    for k in range(AGC):
        cuts.append(cuts[-1] + (SHT - cuts[-1]) // (AGC - k))
    cfg["AG_CUTS"] = cuts
    return cfg


def _chunk_map(ls, cfg):
    """Map half-local row (shard*SH + l) to its chunk-major table row."""
    SH = cfg["SH"]
    cuts = np.asarray(cfg["AG_CUTS"], np.int64) * 128
    s, l = ls // SH, ls % SH
    k = np.searchsorted(cuts, l, side="right") - 1
    width = cuts[k + 1] - cuts[k]
    return 4 * cuts[k] + s * width + (l - cuts[k])


def _host_prep(x, edge_index, batch, cfg):
    """Permute nodes, build per-core padded CSR gather schedules + inputs."""
    N, SH, SHT, HALF, NPAD = (cfg["N"], cfg["SH"], cfg["SHT"], cfg["HALF"],
                              cfg["NPAD"])
    NT, G, PADROW, PADG = cfg["NT"], cfg["G"], cfg["PADROW"], cfg["PADG"]
    src = np.asarray(edge_index[0], dtype=np.int64)
    dst = np.asarray(edge_index[1], dtype=np.int64)
    batch = np.asarray(batch, dtype=np.int64)
    x = np.asarray(x, dtype=np.float32)

    deg = 1 + np.bincount(dst, minlength=N)
    dinv = (1.0 / np.sqrt(deg.astype(np.float64))).astype(np.float32)

    # Greedy balanced half assignment: pick beta[s] to equalize each dst's
    # lo/hi in-edge split (minimizes per-row max(deg_lo, deg_hi), which sets
    # the padded slot count K), subject to the 4*SH shard-capacity cap.
    csr = np.argsort(src, kind="stable")
    sorted_src = src[csr]
    indptr = np.searchsorted(sorted_src, np.arange(N + 1))
    dsts_flat = dst[csr]
    odeg = np.diff(indptr)
    imb = np.zeros(N, np.int32)
    beta = np.zeros(N, np.int8)
    cnt = [0, 0]
    # N/2 per half keeps rows NREAL_SH..SH of every shard as zero padding
    # (PADROW must stay a dummy row)
    capn = N // 2
    for s in np.argsort(-odeg, kind="stable"):
        ds = dsts_flat[indptr[s]:indptr[s + 1]]
        sgn = int(np.sign(imb[ds]).sum())
        if cnt[0] >= capn:
            side = 1
        elif cnt[1] >= capn:
            side = 0
        elif sgn > 0:
            side = 1
        elif sgn < 0:
            side = 0
        else:
            side = 0 if cnt[0] <= cnt[1] else 1
        beta[s] = side
        np.add.at(imb, ds, 1 - 2 * side)
        cnt[side] += 1

    # self-loops excluded: they are added on-device from the local z shard
    deg_lo = np.bincount(dst[beta[src] == 0], minlength=N)
    deg_hi = (deg - 1) - deg_lo

    row_of = np.full(N, -1, np.int64)
    node_of = np.full(NPAD, -1, np.int64)
    mx = np.maximum(deg_lo, deg_hi)
    for h in (0, 1):
        ids = np.nonzero(beta == h)[0]
        order = ids[np.argsort(-mx[ids], kind="stable")]
        k = np.arange(order.size)
        rows = (4 * h + (k % 4)) * SH + k // 4
        row_of[order] = rows
        node_of[rows] = order

    shard_of_row = np.arange(NPAD) // SH
    src_r = row_of[src]
    dst_r = row_of[dst]

    core_rows = []
    counts = np.zeros((NCORES, NT, 128), np.int64)
    for c in range(NCORES):
        p, h = c % 4, c // 4
        m = ((shard_of_row[dst_r] % 4) == p) & (beta[src] == h)
        es, ed = src_r[m], dst_r[m]
        ld = np.where(ed < 4 * SH, ed - p * SH, ed - (p + 4) * SH + SH)
        ls = _chunk_map((es - h * HALF).astype(np.int64), cfg)
        if ls.size:
            assert ls.min() >= 0 and ls.max() < HALF
        order = np.argsort(ld, kind="stable")
        ld, ls = ld[order], ls[order]
        core_rows.append((ld, ls))
        counts[c] = np.bincount(ld, minlength=2 * SH).reshape(NT, 128)

    K = counts.max(axis=(0, 2)).astype(np.int64)
    B = int(K.sum())
    NI = -(-B // CHUNKS_PER_INST)
    B_pad = NI * CHUNKS_PER_INST
    off = np.zeros(NT + 1, np.int64)
    off[1:] = np.cumsum(K)
    blocks = []
    for t in range(NT):
        for k in range(K[t]):
            blocks.append((t, int(k)))

    padrow_m = int(_chunk_map(np.asarray([PADROW], np.int64), cfg)[0])
    idx_cores = []
    for c in range(NCORES):
        ld, ls = core_rows[c]
        slots = np.full(B_pad * 128, padrow_m, np.int64)
        t = ld // 128
        r = ld % 128
        starts = np.searchsorted(ld, ld)
        k = np.arange(ld.size) - starts
        b = off[t] + k
        slots[b * 128 + r] = ls
        spw = SLOTS_PER_INST // 16
        sl = slots.reshape(NI, spw, 16)
        arr16 = sl.transpose(2, 0, 1).reshape(16, NI * spw)
        idx_cores.append(np.tile(arr16, (8, 1)).astype(np.int16))

    def shard_cols(vals, fill):
        out = []
        full = np.full(NPAD, fill, np.float32)
        valid = node_of >= 0
        full[valid] = vals[node_of[valid]]
        for c in range(NCORES):
            sh = full[c * SH:(c + 1) * SH].reshape(SHT, 128).T
            out.append(np.ascontiguousarray(sh, np.float32))
        return out

    dinv_sh = shard_cols(dinv, 0.0)
    cnt_g = np.bincount(batch, minlength=G).astype(np.float32)
    pw_node = (1.0 / np.maximum(cnt_g, 1.0))[batch].astype(np.float32)
    pg_sh = shard_cols(batch.astype(np.float32), float(PADG))
    pw_sh = shard_cols(pw_node, 0.0)

    # premultiply x by dinv: z1*dinv = ((x*dinv) @ W1), so the layer-1 table
    # needs no per-tile scaling on device
    xd = x * dinv[:, None]
    xT_cores = []
    for c in range(NCORES):
        xs = np.zeros((SH, D), np.float32)
        rows = node_of[c * SH:(c + 1) * SH]
        valid = rows >= 0
        xs[valid] = xd[rows[valid]]
        xT_cores.append(np.ascontiguousarray(xs.T.astype(np.float16)))

    xTH = [np.ascontiguousarray(np.concatenate(
        [xT_cores[4 * h + j] for j in range(4)], axis=1))
        for h in (0, 1)]
    dinvH = [np.ascontiguousarray(np.concatenate(
        [dinv_sh[4 * h + j] for j in range(4)], axis=1))
        for h in (0, 1)]

    meta = dict(K=K, B=B, NI=NI, blocks=blocks, node_of=node_of)
    percore = [
        dict(xT=xT_cores[c], xTH=xTH[c // 4], dinvH=dinvH[c // 4],
             idx=idx_cores[c], dinv_sh=dinv_sh[c],
             pg=pg_sh[c], pw=pw_sh[c])
        for c in range(NCORES)
    ]
    return meta, percore


def _build(meta, cfg):
    import concourse.bacc as bacc
    import concourse.mybir as mybir
    import concourse.tile as tile
    from concourse.ap import AP
    from concourse.masks import make_identity

    f32 = mybir.dt.float32
    bf16 = mybir.dt.float16
    i16 = mybir.dt.int16
    i32 = mybir.dt.int32
    Alu = mybir.AluOpType
    Act = mybir.ActivationFunctionType

    N, SH, SHT, HALF = cfg["N"], cfg["SH"], cfg["SHT"], cfg["HALF"]
    NT, NCHUNK, NREAL_SH = cfg["NT"], cfg["NCHUNK"], cfg["NREAL_SH"]
    NI = meta["NI"]
    blocks = meta["blocks"]
    K = meta["K"]

    nc = bacc.Bacc(None, target_bir_lowering=False, num_devices=NCORES,
                   num_swdge_queues=4,
                   dynamic_dma_scratch_size=int(os.environ.get("SCRATCH", "73728")))

    f16in = mybir.dt.float16
    xT_t = nc.dram_tensor("xT", [D, SH], f16in, kind="ExternalInput")
    xTH_t = nc.dram_tensor("xTH", [D, 4 * SH], f16in, kind="ExternalInput")
    idx_t = nc.dram_tensor("idx", [128, NI * (SLOTS_PER_INST // 16)], i16,
                           kind="ExternalInput")
    dinv_t = nc.dram_tensor("dinv_sh", [128, SHT], f32, kind="ExternalInput")
    dinvH_t = nc.dram_tensor("dinvH", [128, 4 * SHT], f32,
                             kind="ExternalInput")
    pg_t = nc.dram_tensor("pg", [128, SHT], f32, kind="ExternalInput")
    pw_t = nc.dram_tensor("pw", [128, SHT], f32, kind="ExternalInput")
    w_ts = [nc.dram_tensor(f"W{i}", [D, D], f32, kind="ExternalInput")
            for i in (1, 2, 3)]
    ga_ts = [nc.dram_tensor(f"gamma{i}", [D, 1], f32, kind="ExternalInput")
             for i in (1, 2, 3)]
    be_ts = [nc.dram_tensor(f"beta{i}", [D, 1], f32, kind="ExternalInput")
             for i in (1, 2, 3)]
    out_t = nc.dram_tensor("out", [D, NCHUNK * 128], f32,
                           kind="ExternalOutput")

    f16 = mybir.dt.float16
    zsh = nc.dram_tensor("zsh", [SH, D], f16)
    table16 = nc.dram_tensor("table16", [HALF, D], f16)
    table = nc.dram_tensor("table", [HALF + 2, 2 * D], f16)
    accp = nc.dram_tensor("accp", [2 * SH, D], f16)
    accs = nc.dram_tensor("accs", [SH, D], f16)
    stat_in = [nc.dram_tensor(f"stat_in{i}", [D, 2], f32) for i in range(3)]
    stat_out = [nc.dram_tensor(f"stat_out{i}", [D, 2], f32,
                               addr_space="Shared") for i in range(3)]
    pool_in = nc.dram_tensor("pool_in", [D, NCHUNK * 128 + 2], f32)
    pool_out = nc.dram_tensor("pool_out", [D, NCHUNK * 128 + 2], f32,
                              addr_space="Shared")

    GRP_PAIR = [[0, 4], [1, 5], [2, 6], [3, 7]]
    GRP_HALF = [[0, 1, 2, 3], [4, 5, 6, 7]]
    GRP_ALL = [list(range(NCORES))]

    with tile.TileContext(nc) as tc:
        with (
            tc.tile_pool(name="const", bufs=1) as cpool,
            tc.tile_pool(name="work", bufs=1) as wpool,
            tc.tile_pool(name="stage", bufs=int(os.environ.get("STAGE_BUFS", "8"))) as spool,
            tc.tile_pool(name="tmp", bufs=2) as tpool,
            tc.tile_pool(name="ps", bufs=1, space="PSUM") as ps,
            tc.tile_pool(name="ps2", bufs=1, space="PSUM") as ps2,
        ):
            # critical-path loads first: everything the layer-1 table build
            # needs; bulk consts (idx table etc.) queue after it
            dinvH_sb = cpool.tile([128, 4 * SHT], f32)
            nc.sync.dma_start(dinvH_sb[:], dinvH_t[:])
            dinv_sb = cpool.tile([128, SHT], f32)
            nc.sync.dma_start(dinv_sb[:], dinv_t[:])
            w_sb = []
            for wt in w_ts:
                w = cpool.tile([D, D], f32, tag=f"w_{wt.name}")
                nc.sync.dma_start(w[:], wt[:])
                wb = cpool.tile([D, D], bf16, tag=f"wb_{wt.name}")
                nc.vector.tensor_copy(wb[:], w[:])
                w_sb.append(wb)
            xT_bf = cpool.tile([D, SH], bf16)
            nc.sync.dma_start(xT_bf[:], xT_t[:])

            def load_consts():
                nc.sync.dma_start(idx_sb[:], idx_t[:])
                nc.sync.dma_start(pg_sb[:], pg_t[:])
                nc.sync.dma_start(pw_sb[:], pw_t[:])
                for g, gt in zip(ga_sb, ga_ts):
                    nc.sync.dma_start(g[:], gt[:])
                for b, bt in zip(be_sb, be_ts):
                    nc.sync.dma_start(b[:], bt[:])

            idx_sb = cpool.tile([128, NI * (SLOTS_PER_INST // 16)], i16)
            pg_sb = cpool.tile([128, SHT], f32)
            pw_sb = cpool.tile([128, SHT], f32)
            ga_sb = [cpool.tile([D, 1], f32, tag=f"g_{gt.name}",
                                name=f"g_{gt.name}") for gt in ga_ts]
            be_sb = [cpool.tile([D, 1], f32, tag=f"b_{bt.name}",
                                name=f"b_{bt.name}") for bt in be_ts]
            ones_sb = cpool.tile([128, 1], bf16)
            nc.vector.memset(ones_sb[:], 1.0)
            ident = cpool.tile([128, 128], f32)
            make_identity(nc, ident[:])
            it_all = cpool.tile([128, NCHUNK * 128], i32)
            nc.gpsimd.iota(it_all[:], pattern=[[1, NCHUNK * 128]], base=0,
                           channel_multiplier=0)
            iota_all = cpool.tile([128, NCHUNK * 128], f32)
            nc.vector.tensor_copy(iota_all[:], it_all[:])

            acc = wpool.tile([128, NT, D], bf16)
            B_sb = wpool.tile([128, SHT, D], f32)
            B_bf = wpool.tile([128, SHT, D], bf16)
            z_sb = wpool.tile([128, SHT, D], bf16)

            zsh_v = zsh[:].rearrange("(t p) d -> p t d", p=128)
            accp_v = accp[:].rearrange("(t p) d -> p t d", p=128)
            accs_v = accs[:].rearrange("(t p) d -> p t d", p=128)

            # AllGather in chunks (table is chunk-major so outputs are
            # contiguous), pipelined behind the per-tile z-writes
            ag_cuts = cfg["AG_CUTS"]
            AGC = len(ag_cuts) - 1

            def layer_z_write(layer, src_tiles):
                for k in range(AGC):
                    t0, t1 = ag_cuts[k], ag_cuts[k + 1]
                    for g0 in range(t0, t1, 8):
                        gn = min(8, t1 - g0)
                        pzb = ps.tile([128, 8, D], f32, tag="pzbig",
                                      space="PSUM")
                        for tt in range(gn):
                            nc.tensor.matmul(pzb[:, tt, :],
                                             lhsT=src_tiles(g0 + tt),
                                             rhs=w_sb[layer][:], start=True,
                                             stop=True)
                        dbc = dinv_sb[:, g0:g0 + gn].unsqueeze(2)\
                            .broadcast_to([128, gn, D])
                        nc.vector.tensor_tensor(
                            out=z_sb[:, g0:g0 + gn, :], in0=pzb[:, 0:gn, :],
                            in1=dbc, op=Alu.mult)
                    nc.sync.dma_start(zsh_v[:, t0:t1, :], z_sb[:, t0:t1, :])
                    r0, r1 = 4 * t0 * 128, 4 * t1 * 128
                    nc.gpsimd.collective_compute(
                        "AllGather", Alu.bypass, replica_groups=GRP_HALF,
                        ins=[zsh[t0 * 128:t1 * 128, :]],
                        outs=[table16[r0:r1, :]])
                    nc.sync.dma_start(table[r0:r1, 0:D], table16[r0:r1, :])

            def layer1_local_table():
                """Each core computes the whole half's z1 from xTH (already
                dinv-scaled on host) — no AllGather/barrier gates the first
                gather. Matmuls land 8-at-a-time in one PSUM region with one
                batched copy out."""
                for k in range(AGC):
                    t0, t1 = ag_cuts[k], ag_cuts[k + 1]
                    nt = t1 - t0
                    for j in range(4):
                        xst = tpool.tile([D, 25 * 128], bf16, tag="xst")
                        nc.sync.dma_start(
                            xst[:, 0:nt * 128],
                            xTH_t[:, j * SH + t0 * 128:j * SH + t1 * 128])
                        zst = tpool.tile([128, 25, D], bf16, tag="zst")
                        for g0 in range(0, nt, 8):
                            gn = min(8, nt - g0)
                            pzb = ps.tile([128, 8, D], f32, tag="pzbig",
                                          space="PSUM")
                            for tt in range(gn):
                                nc.tensor.matmul(
                                    pzb[:, tt, :],
                                    lhsT=xst[:, (g0 + tt) * 128:
                                             (g0 + tt + 1) * 128],
                                    rhs=w_sb[0][:], start=True, stop=True)
                            nc.scalar.copy(zst[:, g0:g0 + gn, :],
                                           pzb[:, 0:gn, :])
                        base = (4 * t0 + j * nt) * 128
                        tab_v = table[base:base + nt * 128, 0:D].rearrange(
                            "(t p) d -> p t d", p=128)
                        nc.sync.dma_start(tab_v, zst[:, 0:nt, :])
                # own-shard z*dinv for the self-loop add after RS (xT is
                # dinv-scaled on host too)
                for g0 in range(0, SHT, 8):
                    gn = min(8, SHT - g0)
                    pzb = ps.tile([128, 8, D], f32, tag="pzbig",
                                  space="PSUM")
                    for tt in range(gn):
                        t = g0 + tt
                        nc.tensor.matmul(pzb[:, tt, :],
                                         lhsT=xT_bf[:, t * 128:(t + 1) * 128],
                                         rhs=w_sb[0][:], start=True,
                                         stop=True)
                    nc.scalar.copy(z_sb[:, g0:g0 + gn, :], pzb[:, 0:gn, :])

            def gather_agg():
                if os.environ.get("NO_ADDS"):
                    nc.vector.memset(acc[:], 0.0)
                for t in range(NT):
                    if K[t] == 0:
                        nc.vector.memset(acc[:, t, :], 0.0)
                for i in range(NI):
                    st = spool.tile([128, CHUNKS_PER_INST, 2 * D], bf16,
                                    tag="stage")
                    nc.gpsimd.dma_gather(
                        out_ap=st[:],
                        in_ap=table[0:HALF, :],
                        idxs_ap=idx_sb[:, i * (SLOTS_PER_INST // 16):
                                       (i + 1) * (SLOTS_PER_INST // 16)],
                        num_idxs=SLOTS_PER_INST,
                        num_idxs_reg=SLOTS_PER_INST,
                        elem_size=2 * D,
                        queue_num=i % 4,
                    )
                    if os.environ.get("NO_ADDS"):
                        continue
                    c0 = 0
                    while c0 < CHUNKS_PER_INST:
                        b = i * CHUNKS_PER_INST + c0
                        if b >= len(blocks):
                            break
                        t, k0 = blocks[b]
                        m = 1
                        while (c0 + m < CHUNKS_PER_INST
                               and i * CHUNKS_PER_INST + c0 + m < len(blocks)
                               and blocks[i * CHUNKS_PER_INST + c0 + m][0] == t):
                            m += 1
                        mm = m
                        while mm > 2:
                            h = mm // 2
                            nc.vector.tensor_tensor(
                                out=st[:, c0:c0 + h, 0:D],
                                in0=st[:, c0:c0 + h, 0:D],
                                in1=st[:, c0 + mm - h:c0 + mm, 0:D],
                                op=Alu.add)
                            mm -= h
                        if k0 == 0:
                            if mm == 2:
                                nc.vector.tensor_tensor(
                                    out=acc[:, t, :], in0=st[:, c0, 0:D],
                                    in1=st[:, c0 + 1, 0:D], op=Alu.add)
                            else:
                                nc.scalar.copy(acc[:, t, :], st[:, c0, 0:D])
                        else:
                            if mm == 2:
                                nc.vector.tensor_tensor(
                                    out=st[:, c0, 0:D], in0=st[:, c0, 0:D],
                                    in1=st[:, c0 + 1, 0:D], op=Alu.add)
                            nc.vector.tensor_add(acc[:, t, :], acc[:, t, :],
                                                 st[:, c0, 0:D])
                        c0 += m

            def reduce_pair():
                nc.sync.dma_start(accp_v, acc[:])
                nc.gpsimd.collective_compute(
                    "ReduceScatter", Alu.add, replica_groups=GRP_PAIR,
                    ins=[accp[:]], outs=[accs[:]])

            def load_B_and_stats(li):
                ps_sum_t = ps.tile([D, 1], f32, tag="ps_sum", space="PSUM")
                ps_sq_t = ps.tile([D, 1], f32, tag="ps_sq", space="PSUM")
                ps_sum, ps_sq = ps_sum_t[:], ps_sq_t[:]
                nc.sync.dma_start(B_bf[:], accs_v[:, :, :])
                # self-loop term: z_sb already holds z*dinv for own shard
                nc.vector.tensor_tensor(
                    out=B_sb[:].rearrange("p t d -> p (t d)"),
                    in0=B_bf[:].rearrange("p t d -> p (t d)"),
                    in1=z_sb[:].rearrange("p t d -> p (t d)"), op=Alu.add)
                dbc = dinv_sb[:].unsqueeze(2).broadcast_to([128, SHT, D])
                nc.vector.tensor_tensor(out=B_sb[:], in0=B_sb[:], in1=dbc,
                                        op=Alu.mult)
                nc.vector.tensor_copy(
                    B_bf[:].rearrange("p t d -> p (t d)"),
                    B_sb[:].rearrange("p t d -> p (t d)"))
                sq = wpool.tile([128, SHT, D], bf16, tag="sqbig")
                nc.scalar.square(
                    sq[:].rearrange("p t d -> p (t d)"),
                    B_bf[:].rearrange("p t d -> p (t d)"))
                for t in range(SHT):
                    nc.tensor.matmul(ps_sum, lhsT=B_bf[:, t, :],
                                     rhs=ones_sb[:], start=(t == 0),
                                     stop=(t == SHT - 1))
                    nc.tensor.matmul(ps_sq, lhsT=sq[:, t, :],
                                     rhs=ones_sb[:],
                                     start=(t == 0), stop=(t == SHT - 1))
                stt = tpool.tile([D, 2], f32, tag="stt")
                nc.scalar.copy(stt[:, 0:1], ps_sum)
                nc.scalar.copy(stt[:, 1:2], ps_sq)
                if li == 2:
                    nc.sync.dma_start(pool_in[:, NCHUNK * 128:], stt[:])
                else:
                    nc.sync.dma_start(stat_in[li][:], stt[:])
                    nc.gpsimd.collective_compute(
                        "AllReduce", Alu.add, replica_groups=GRP_ALL,
                        ins=[stat_in[li][:]], outs=[stat_out[li][:]])

            def bn_params(li):
                st = tpool.tile([D, 2], f32, tag="st2")
                if li == 2:
                    nc.sync.dma_start(st[:], pool_out[:, NCHUNK * 128:])
                else:
                    nc.sync.dma_start(st[:], stat_out[li][:])
                mean = tpool.tile([D, 1], f32, tag="mean")
                nc.scalar.mul(mean[:], st[:, 0:1], 1.0 / N)
                ex2 = tpool.tile([D, 1], f32, tag="ex2")
                nc.scalar.mul(ex2[:], st[:, 1:2], 1.0 / N)
                var = tpool.tile([D, 1], f32, tag="var")
                nc.vector.tensor_mul(var[:], mean[:], mean[:])
                nc.vector.tensor_tensor(out=var[:], in0=ex2[:], in1=var[:],
                                        op=Alu.subtract)
                nc.vector.tensor_scalar_add(var[:], var[:], EPS)
                rv = tpool.tile([D, 1], f32, tag="rv")
                nc.vector.reciprocal(rv[:], var[:])
                rstd = tpool.tile([D, 1], f32, tag="rstd")
                nc.scalar.sqrt(rstd[:], rv[:])
                a = tpool.tile([D, 1], f32, tag=f"a{li}")
                nc.vector.tensor_mul(a[:], ga_sb[li][:], rstd[:])
                cc = tpool.tile([D, 1], f32, tag=f"c{li}")
                nc.vector.tensor_mul(cc[:], a[:], mean[:])
                nc.vector.tensor_tensor(out=cc[:], in0=be_sb[li][:], in1=cc[:],
                                        op=Alu.subtract)
                return a, cc

            ht_all = wpool.tile([D, SHT, 128], bf16)

            def pre_transpose():
                # transposes need no BN params: run them under the stats
                # AllReduce
                for g0 in range(0, SHT, 4):
                    gn = min(4, SHT - g0)
                    ptb = ps2.tile([D, 4, 128], f32, tag="ptb", space="PSUM")
                    for tt in range(gn):
                        nc.tensor.transpose(ptb[:, tt, :],
                                            B_sb[:, g0 + tt, :], ident[:])
                    nc.scalar.copy(ht_all[:, g0:g0 + gn, :], ptb[:, 0:gn, :])

            def apply_bn(a, cc):
                for g0 in range(0, SHT, 8):
                    gn = min(8, SHT - g0)
                    nc.scalar.activation(ht_all[:, g0:g0 + gn, :],
                                         ht_all[:, g0:g0 + gn, :], Act.Relu,
                                         bias=cc[:], scale=a[:])
                d0 = NREAL_SH - (SHT - 1) * 128
                if 0 < d0 < 128:
                    nc.vector.memset(ht_all[:, SHT - 1, d0:], 0.0)

            # ================= layers =================
            layer1_local_table()
            load_consts()
            gather_agg()
            reduce_pair()
            load_B_and_stats(0)
            pre_transpose()
            a1, c1 = bn_params(0)
            apply_bn(a1, c1)

            layer_z_write(1, lambda t: ht_all[:, t, :])
            gather_agg()
            reduce_pair()
            load_B_and_stats(1)
            pre_transpose()
            a2, c2 = bn_params(1)
            apply_bn(a2, c2)

            layer_z_write(2, lambda t: ht_all[:, t, :])
            gather_agg()
            reduce_pair()
            load_B_and_stats(2)

            ps_pool = [ps.tile([128, D], f32, tag=f"pool{q}", name=f"pool{q}",
                               space="PSUM") for q in range(NCHUNK)]
            for t in range(SHT):
                eq = tpool.tile([128, NCHUNK, 128], bf16, tag="eq")
                nc.vector.tensor_scalar(
                    out=eq[:], in0=iota_all[:].rearrange(
                        "p (q c) -> p q c", c=128),
                    scalar1=pg_sb[:, t:t + 1], scalar2=pw_sb[:, t:t + 1],
                    op0=Alu.is_equal, op1=Alu.mult)
                for q in range(NCHUNK):
                    nc.tensor.matmul(ps_pool[q][:], lhsT=eq[:, q, :],
                                     rhs=B_bf[:, t, :], start=(t == 0),
                                     stop=(t == SHT - 1))
            poolT = wpool.tile([D, NCHUNK * 128], f32)
            pc = tpool.tile([128, NCHUNK, D], f32, tag="poolc")
            for q in range(NCHUNK):
                nc.scalar.copy(pc[:, q, :], ps_pool[q][:])
            ptb2 = ps2.tile([D, 4, 128], f32, tag="ptb", space="PSUM")
            for q in range(NCHUNK):
                nc.tensor.transpose(ptb2[:, q, :], pc[:, q, :], ident[:])
            nc.scalar.copy(poolT[:].rearrange("d (q c) -> d q c", c=128),
                           ptb2[:])
            nc.sync.dma_start(pool_in[:, 0:NCHUNK * 128], poolT[:])
            nc.gpsimd.collective_compute(
                "AllReduce", Alu.add, replica_groups=GRP_ALL,
                ins=[pool_in[:]], outs=[pool_out[:]])
            a3, c3 = bn_params(2)
            pool_sb = wpool.tile([D, NCHUNK * 128], f32)
            nc.sync.dma_start(pool_sb[:], pool_out[:, 0:NCHUNK * 128])
            out_sb = wpool.tile([D, NCHUNK * 128], f32)
            nc.scalar.activation(out_sb[:], pool_sb[:], Act.Identity,
                                 bias=c3[:], scale=a3[:])
            nc.sync.dma_start(out_t[:], out_sb[:])

    nc.compile()
    return nc


def run(inputs, cfg, trace=False, trace_cores=None):
    from concourse.bass_utils import run_bass_kernel_spmd

    x = np.asarray(inputs["x"], np.float32)
    edge_index = np.asarray(inputs["edge_index"])
    batch = np.asarray(inputs["batch"])

    meta, percore = _host_prep(x, edge_index, batch, cfg)
    nc = _build(meta, cfg)

    in_maps = []
    for c in range(NCORES):
        m = dict(percore[c])
        for i in (1, 2, 3):
            m[f"W{i}"] = np.asarray(inputs[f"W{i}"], np.float32)
            m[f"gamma{i}"] = np.asarray(inputs[f"gamma{i}"],
                                        np.float32).reshape(D, 1)
            m[f"beta{i}"] = np.asarray(inputs[f"beta{i}"],
                                       np.float32).reshape(D, 1)
        in_maps.append(m)

    kw = {}
    if trace:
        kw = dict(trace=True, trace_cores=trace_cores or [0])
    res = run_bass_kernel_spmd(nc, in_maps, list(range(NCORES)), **kw)
    out = res.results[0]["out"]  # [D, NCHUNK*128]
    return np.ascontiguousarray(out[:, :cfg["G"]].T), res


def kernel(**inputs):
    cfg = make_cfg(50000, 500, 49)
    out, _ = run(inputs, cfg)
    return out

